# revision 6
# baseline (speedup 1.0000x reference)
"""Trainium2 Bass kernel for nn_CustomLoss_45449343926664 (retrieval_knn).

loss = mse(mean(c1), mean(c2))
     + mean_i min_j ||c1_i - c2_j||^2
     + mean_k relu(0.1 - var(c1)_k)

Sharding (2D over the 8 cores): core c = (a, b), a = c//2 in 0..3 an
i-block of 2048 cluster1 rows, b = c%2 a half of cluster2 (4096 rows).
Each core computes its [2048, 4096] block of the distance matrix on the
tensor engine in bf16 (c1 pre-scaled by 2 so PSUM holds 2<c1,c2>), with
matmuls in "j-on-partitions" orientation: psum tile [128 j, 2048 i] per
j-tile, so -|c2_j|^2 is a per-partition bias. The drain/row-max of the
8192x8192 elements is split across DVE and ACT:

  - DVE j-tiles: scalar_tensor_tensor fused drain
        zD' = max(psum + bias, zD)     (one 1x pass, ping-pong accum)
  - ACT j-tiles: activation(Identity, bias) -> z bf16 tile; pairs of z
    tiles are folded by one DVE bf16 tensor_max (2x mode) into zA.
  - tail: zfin = max(zD, zA halves); 16 PE transposes + two 3D
    reduce_max give per-row max over the core's j-half; the host maxes
    the two j-halves per i-block.

|c1_i|^2 (fp32) and the mean/variance column stats (fp32 ones-matmuls
accumulated in PSUM) run on disjoint 1024-row slices per core; the host
only combines the tiny per-core partials (a few KB) into the scalar.

Host-side input prep per core: slicing, the bf16 cast + transpose of the
matmul operands (layout prep), and |c2_j|^2 of the bf16-rounded c2 (32KB,
consistent with the bf16 cross term).
"""
import os
import sys

import numpy as np
import ml_dtypes

if os.path.isdir("/opt/trn_rl_repo") and "/opt/trn_rl_repo" not in sys.path:
    sys.path.insert(0, "/opt/trn_rl_repo")

from contextlib import ExitStack

import concourse.bass as bass
import concourse.tile as tile
from concourse import bacc, mybir
from concourse.bass_utils import run_bass_kernel_spmd
from concourse.masks import make_identity

F32 = mybir.dt.float32
BF16 = mybir.dt.bfloat16
BF16_NP = ml_dtypes.bfloat16
NEG_BIG = -3.0e38

N_CORES = 8
N1 = 8192            # cluster1 rows (total)
N2 = 8192            # cluster2 rows
D = 128              # feature dim = partition count
P = 128
NIB = 4              # i-blocks (a axis)
NJB = 2              # j-halves (b axis)
NI = N1 // NIB       # 2048 c1 rows per core
NJ = N2 // NJB       # 4096 c2 rows per core
MTI = NI // P        # 16 i-tiles of 128
NJT = NJ // P        # 32 j-tiles of 128 per core
NST = N1 // N_CORES  # 1024 stats rows per core
NCHUNK = 8           # c2bT DMA chunks (4 j-tiles each)
JT_PER_CHUNK = NJT // NCHUNK

# j-tiles drained by the fused DVE path (7 of 32); rest go to ACT.
DVE_TILES = {2, 7, 12, 17, 22, 27, 30}
assert len(DVE_TILES) == 7
MIN_VARIANCE = 0.1

_cached = {}


def _build_program():
    """Build + compile the single-core SPMD program (same for all cores)."""
    nc = bacc.Bacc(
        "TRN2",
        target_bir_lowering=False,
        debug=False,
        enable_asserts=False,
        num_devices=N_CORES,
    )

    d_c1st = nc.dram_tensor("c1st", [NST, D], F32, kind="ExternalInput").ap()
    d_c2st = nc.dram_tensor("c2st", [NST, D], F32, kind="ExternalInput").ap()
    d_c1bT = nc.dram_tensor("c1bT", [D, NI], BF16, kind="ExternalInput").ap()
    d_c2bT = nc.dram_tensor("c2bT", [D, NJ], BF16, kind="ExternalInput").ap()
    d_sq2neg = nc.dram_tensor("sq2neg", [P, NJT], F32, kind="ExternalInput").ap()

    d_gmax = nc.dram_tensor("gmax", [P, MTI], F32, kind="ExternalOutput").ap()
    d_sq1 = nc.dram_tensor("sq1", [P, NST // P], F32, kind="ExternalOutput").ap()
    d_stats = nc.dram_tensor("stats", [3, D], F32, kind="ExternalOutput").ap()

    with tile.TileContext(nc) as tc, ExitStack() as ctx:
        const = ctx.enter_context(tc.tile_pool(name="const", bufs=1))
        c2pool = ctx.enter_context(tc.tile_pool(name="c2pool", bufs=NCHUNK))
        zring = ctx.enter_context(tc.tile_pool(name="zring", bufs=5))
        psum = ctx.enter_context(tc.tile_pool(name="psum", bufs=2, space="PSUM"))

        t_c1st = const.tile([P, NST // P, P], F32)
        t_c2st = const.tile([P, NST // P, P], F32)
        t_c1bT = const.tile([P, NI], BF16)
        t_sq2neg = const.tile([P, NJT], F32)
        t_ones = const.tile([P, 1], F32)
        t_sq1 = const.tile([P, NST // P], F32)
        t_zA = [const.tile([P, 2, NI], BF16, name=f"zA{i}") for i in range(2)]
        t_zD = [const.tile([P, NI], BF16, name=f"zD{i}") for i in range(2)]
        t_zfin = const.tile([P, NI], BF16)
        t_gmax = const.tile([P, MTI], F32)
        t_c1sq = const.tile([P, NST // P, P], F32)
        t_souts = const.tile([1, 3, D], F32)
        t_sttscratch = const.tile([P, P], F32)
        t_ident = const.tile([P, P], BF16)
        t_dummy = const.tile([P, 1], F32)

        # identity early (gpsimd) so PE warm-up matmuls can start during loads
        make_identity(nc, t_ident[:])
        nc.gpsimd.memset(t_zA[0][:], NEG_BIG)
        nc.gpsimd.memset(t_zD[0][:], NEG_BIG)
        nc.vector.memset(t_ones[:], 1.0)

        # ---- input DMAs, spread across the three DGE-capable engines ----
        nc.scalar.dma_start(t_c1bT[:], d_c1bT)
        nc.sync.dma_start(t_sq2neg[:], d_sq2neg)
        t_c2bT = []
        dma_engs = [nc.scalar, nc.sync, nc.gpsimd]
        for ci in range(NCHUNK):
            t = c2pool.tile([P, JT_PER_CHUNK, P], BF16, name=f"c2bT{ci}")
            dma_engs[ci % 3].dma_start(
                t[:],
                d_c2bT[:, ci * JT_PER_CHUNK * P : (ci + 1) * JT_PER_CHUNK * P]
                .rearrange("k (t p) -> k t p", p=P),
            )
            t_c2bT.append(t)
        nc.sync.dma_start(t_c1st[:], d_c1st.rearrange("(t p) k -> p t k", p=P))
        nc.gpsimd.dma_start(t_c2st[:], d_c2st.rearrange("(t p) k -> p t k", p=P))

        # warm the ACT table set before the drain path needs it
        nc.scalar.activation(t_dummy[:], t_ones[:],
                             mybir.ActivationFunctionType.Identity, bias=0.0)

        # PE warm-up: keep HAM busy while inputs stream in
        pwarm = psum.tile([P, P], F32, tag="pcross", name="pwarm")
        for w in range(24):
            nc.tensor.matmul(pwarm[:], t_ident[:], t_ident[:],
                             start=(w == 0), stop=(w == 23))

        # ---- |c1_i|^2 (fp32, stats slice rows) ----
        for t in range(NST // P):
            nc.vector.scalar_tensor_tensor(
                out=t_sttscratch[:],
                in0=t_c1st[:, t],
                scalar=1.0,
                in1=t_c1st[:, t],
                op0=mybir.AluOpType.mult,
                op1=mybir.AluOpType.mult,
                accum_out=t_sq1[:, t : t + 1],
            )

        # ---- cross matmuls (j on partitions) + dual-engine drain ----
        nd = na = 0
        zhalf = 0
        zt = None
        for t in range(NJT):
            pt = psum.tile([P, NI], F32, tag="pcross", name="pcross")
            lhsT = t_c2bT[t // JT_PER_CHUNK][:, t % JT_PER_CHUNK]
            for c in range(NI // 512):
                nc.tensor.matmul(
                    pt[:, c * 512 : (c + 1) * 512],
                    lhsT,
                    t_c1bT[:, c * 512 : (c + 1) * 512],
                    start=True,
                    stop=True,
                )
            bias = t_sq2neg[:, t : t + 1]
            if t in DVE_TILES:
                nc.vector.scalar_tensor_tensor(
                    out=t_zD[(nd + 1) % 2][:],
                    in0=pt[:],
                    scalar=bias,
                    in1=t_zD[nd % 2][:],
                    op0=mybir.AluOpType.add,
                    op1=mybir.AluOpType.max,
                )
                nd += 1
            else:
                if zhalf == 0:
                    zt = zring.tile([P, 2, NI], BF16, name="zt")
                nc.scalar.activation(
                    zt[:, zhalf], pt[:], mybir.ActivationFunctionType.Identity,
                    bias=bias, scale=1.0,
                )
                if zhalf == 1:
                    # one bf16 2x tensor_max folds both tiles of the pair
                    # into the two independent halves of the zA accumulator
                    nc.vector.tensor_max(t_zA[(na + 1) % 2][:],
                                         t_zA[na % 2][:], zt[:])
                    na += 1
                zhalf ^= 1
        if zhalf == 1:  # lone last ACT tile: pad its pair-half with -inf
            nc.gpsimd.memset(zt[:, 1], NEG_BIG)
            nc.vector.tensor_max(t_zA[(na + 1) % 2][:],
                                 t_zA[na % 2][:], zt[:])
            na += 1

        # ---- column stats: sum(c1), sum(c1^2), sum(c2 slice) (fp32 MMs) ----
        nc.scalar.activation(t_c1sq[:], t_c1st[:], mybir.ActivationFunctionType.Square)
        ps = psum.tile([1, 3, D], F32, tag="pcross", name="pstats")
        for t in range(NST // P):
            nc.tensor.matmul(ps[:, 0], t_ones[:], t_c1st[:, t],
                             start=(t == 0), stop=(t == NST // P - 1))
        for t in range(NST // P):
            nc.tensor.matmul(ps[:, 1], t_ones[:], t_c1sq[:, t],
                             start=(t == 0), stop=(t == NST // P - 1))
        for t in range(NST // P):
            nc.tensor.matmul(ps[:, 2], t_ones[:], t_c2st[:, t],
                             start=(t == 0), stop=(t == NST // P - 1))
        nc.vector.tensor_copy(t_souts[:], ps[:])
        nc.sync.dma_start(d_stats, t_souts[0])

        # ---- tail: combine partial maxes, partition-reduce via PE transpose
        nc.vector.tensor_max(t_zfin[:], t_zD[nd % 2][:], t_zA[na % 2][:, 0])
        nc.vector.tensor_max(t_zfin[:], t_zfin[:], t_zA[na % 2][:, 1])
        for h in range(2):
            ptr = psum.tile([P, 8, P], BF16, tag="pcross", name="ptr")
            for c in range(8):
                nc.tensor.transpose(
                    ptr[:, c], t_zfin[:, (h * 8 + c) * P : (h * 8 + c + 1) * P],
                    t_ident[:])
            nc.vector.tensor_reduce(t_gmax[:, h * 8 : (h + 1) * 8], ptr[:],
                                    axis=mybir.AxisListType.X,
                                    op=mybir.AluOpType.max)
        nc.sync.dma_start(d_gmax, t_gmax[:])
        nc.sync.dma_start(d_sq1, t_sq1[:])

    nc.compile()
    return nc


def _prep_inputs(cluster1: np.ndarray, cluster2: np.ndarray):
    """Host-side sharding + operand layout prep."""
    c2b = cluster2.astype(BF16_NP)
    c2bT = np.ascontiguousarray(c2b.T)                       # [128, 8192] bf16
    sq2 = (c2b.astype(np.float32) ** 2).sum(axis=1)          # [8192] fp32

    in_maps = []
    for c in range(N_CORES):
        a, b = divmod(c, NJB)
        c1s = np.ascontiguousarray(cluster1[a * NI : (a + 1) * NI])
        c1bT = np.ascontiguousarray((2.0 * c1s).astype(BF16_NP).T)  # [128, 2048]
        sq2h = sq2[b * NJ : (b + 1) * NJ]
        sq2neg = np.ascontiguousarray((-sq2h).reshape(NJT, P).T).astype(np.float32)
        in_maps.append({
            "c1st": np.ascontiguousarray(cluster1[c * NST : (c + 1) * NST]),
            "c2st": np.ascontiguousarray(cluster2[c * NST : (c + 1) * NST]),
            "c1bT": c1bT,
            "c2bT": np.ascontiguousarray(c2bT[:, b * NJ : (b + 1) * NJ]),
            "sq2neg": sq2neg,
        })
    return in_maps


def _finish(results) -> np.float32:
    """Combine the 8 per-core partials into the scalar loss (host, fp64)."""
    dist_sum = 0.0
    s1 = np.zeros(D, np.float64)
    q1 = np.zeros(D, np.float64)
    s2 = np.zeros(D, np.float64)
    for a in range(NIB):
        g0 = np.asarray(results[a * NJB + 0]["gmax"], np.float64)  # [128, 16]
        g1 = np.asarray(results[a * NJB + 1]["gmax"], np.float64)
        gmax = np.maximum(g0, g1)                # max over both j-halves
        # rows of block a: i = t*128 + p  -> per-row |c1_i|^2 from the two
        # stats slices covering this block (cores 2a and 2a+1)
        sq1 = np.concatenate([
            np.asarray(results[a * NJB + 0]["sq1"], np.float64),
            np.asarray(results[a * NJB + 1]["sq1"], np.float64)], axis=1)
        dist_sum += (sq1 - gmax).sum()
    for r in results:
        stats = np.asarray(r["stats"], np.float64)  # [3, 128]
        s1 += stats[0]
        q1 += stats[1]
        s2 += stats[2]
    dist = dist_sum / N1
    m1 = s1 / N1
    m2 = s2 / N2
    mean_loss = ((m1 - m2) ** 2).mean()
    var = q1 / N1 - m1 ** 2
    disp = np.maximum(MIN_VARIANCE - var, 0.0).mean()
    return np.float32(mean_loss + dist + disp)


def _run(inputs, trace=False, **kwargs):
    """Run on the 8 NeuronCores. Returns (loss_scalar, BassKernelResults)."""
    if "nc" not in _cached:
        _cached["nc"] = _build_program()
    nc = _cached["nc"]
    in_maps = _prep_inputs(np.asarray(inputs["cluster1"], np.float32),
                           np.asarray(inputs["cluster2"], np.float32))
    res = run_bass_kernel_spmd(nc, in_maps, list(range(N_CORES)), trace=trace,
                               **kwargs)
    loss = _finish(res.results)
    return loss, res


def kernel(cluster1: np.ndarray, cluster2: np.ndarray) -> np.ndarray:
    loss, _ = _run({"cluster1": cluster1, "cluster2": cluster2})
    return np.asarray(loss, dtype=np.float32)


# revision 7
# speedup vs baseline: 1.1026x; 1.1026x over previous
"""Trainium2 Bass kernel for nn_CustomLoss_45449343926664 (retrieval_knn).

loss = mse(mean(c1), mean(c2))
     + mean_i min_j ||c1_i - c2_j||^2
     + mean_k relu(0.1 - var(c1)_k)

Sharding: cluster1 rows are data-parallel across the 8 cores (1024 rows
each); cluster2 is replicated. Each core computes its [1024, 8192] block
of the distance matrix on the tensor engine in bf16 (c1 pre-scaled by 2
so PSUM holds 2<c1,c2>), with matmuls in "j-on-partitions" orientation:
psum tile [128 j, 1024 i] per j-tile, so -|c2_j|^2 is a per-partition
bias. The 8192x8192-element drain/row-max is split across DVE and ACT:

  - DVE j-tiles: scalar_tensor_tensor fused drain
        zD' = max(psum + bias, zD)     (one 1x pass, ping-pong accum)
  - ACT j-tiles: activation(Identity, bias) -> z bf16 tile; PAIRS of z
    tiles are folded by one DVE bf16 tensor_max (2x mode, [128, 2048])
    into the two independent halves of the zA accumulator.
  - tail: zfin = max(zD, zA halves); 8 PE transposes + one 3D
    reduce_max give per-row max_j(2<c1_i,c2_j> - |c2_j|^2).

|c1_i|^2 (fp32) and the mean/variance column stats (fp32 ones-matmuls
accumulated in PSUM) are computed on device as well; the host only sums
the 8 tiny per-core partials (a few KB) into the final scalar.

Host-side input prep per core: slicing, the bf16 cast + transpose of the
matmul operands (layout prep), and |c2_j|^2 of the bf16-rounded c2 (32KB,
consistent with the bf16 cross term).
"""
import os
import sys

import numpy as np
import ml_dtypes

if os.path.isdir("/opt/trn_rl_repo") and "/opt/trn_rl_repo" not in sys.path:
    sys.path.insert(0, "/opt/trn_rl_repo")

from contextlib import ExitStack

import concourse.bass as bass
import concourse.tile as tile
from concourse import bacc, mybir
from concourse.bass_utils import run_bass_kernel_spmd
from concourse.masks import make_identity

F32 = mybir.dt.float32
BF16 = mybir.dt.bfloat16
BF16_NP = ml_dtypes.bfloat16
NEG_BIG = -3.0e38

N_CORES = 8
N1 = 8192            # cluster1 rows (total)
N2 = 8192            # cluster2 rows
D = 128              # feature dim = partition count
P = 128
NI = N1 // N_CORES   # 1024 c1 rows per core
MTI = NI // P        # 8 i-tiles of 128
NJT = N2 // P        # 64 j-tiles of 128
NCHUNK = 8           # c2bT DMA chunks
JT_PER_CHUNK = NJT // NCHUNK

# j-tiles drained by the fused DVE path (17 of 64); the other 47 go to
# ACT (one of them padded with -inf to make 24 fold-pairs).
DVE_TILES = {2, 6, 10, 14, 18, 22, 26, 30, 34, 38, 42, 46, 50, 54, 58, 61, 63}
MIN_VARIANCE = 0.1

_cached = {}


def _build_program():
    """Build + compile the single-core SPMD program (same for all cores)."""
    nc = bacc.Bacc(
        "TRN2",
        target_bir_lowering=False,
        debug=False,
        enable_asserts=False,
        num_devices=N_CORES,
    )

    d_c1s = nc.dram_tensor("c1s", [NI, D], F32, kind="ExternalInput").ap()
    d_c2s = nc.dram_tensor("c2s", [NI, D], F32, kind="ExternalInput").ap()
    d_c1bT = nc.dram_tensor("c1bT", [D, NI], BF16, kind="ExternalInput").ap()
    d_c2bT = nc.dram_tensor("c2bT", [D, N2], BF16, kind="ExternalInput").ap()
    d_sq2neg = nc.dram_tensor("sq2neg", [P, NJT], F32, kind="ExternalInput").ap()

    d_gmax = nc.dram_tensor("gmax", [P, MTI], F32, kind="ExternalOutput").ap()
    d_sq1 = nc.dram_tensor("sq1", [P, MTI], F32, kind="ExternalOutput").ap()
    d_stats = nc.dram_tensor("stats", [3, D], F32, kind="ExternalOutput").ap()

    with tile.TileContext(nc) as tc, ExitStack() as ctx:
        const = ctx.enter_context(tc.tile_pool(name="const", bufs=1))
        c2pool = ctx.enter_context(tc.tile_pool(name="c2pool", bufs=NCHUNK))
        zring = ctx.enter_context(tc.tile_pool(name="zring", bufs=5))
        psum = ctx.enter_context(tc.tile_pool(name="psum", bufs=4, space="PSUM"))

        t_c1s = const.tile([P, MTI, P], F32)
        t_c2s = const.tile([P, MTI, P], F32)
        t_c1bT = const.tile([P, NI], BF16)
        t_sq2neg = const.tile([P, NJT], F32)
        t_ones = const.tile([P, 1], F32)
        t_sq1 = const.tile([P, MTI], F32)
        t_zA = [const.tile([P, 2, NI], BF16, name=f"zA{i}") for i in range(2)]
        t_zD = [const.tile([P, NI], BF16, name=f"zD{i}") for i in range(2)]
        t_zfin = const.tile([P, NI], BF16)
        t_gmax = const.tile([P, MTI], F32)
        t_c1sq = const.tile([P, MTI, P], F32)
        t_souts = const.tile([1, 3, D], F32)
        t_sttscratch = const.tile([P, P], F32)
        t_ident = const.tile([P, P], BF16)
        t_dummy = const.tile([P, 1], F32)

        # identity early (gpsimd) so PE warm-up matmuls can start during loads
        make_identity(nc, t_ident[:])
        nc.gpsimd.memset(t_zA[0][:], NEG_BIG)
        nc.gpsimd.memset(t_zD[0][:], NEG_BIG)
        nc.vector.memset(t_ones[:], 1.0)

        # ---- input DMAs, spread across the three DGE-capable engines ----
        nc.scalar.dma_start(t_c1bT[:], d_c1bT)
        nc.sync.dma_start(t_sq2neg[:], d_sq2neg)
        t_c2bT = []
        dma_engs = [nc.scalar, nc.sync, nc.gpsimd]
        for ci in range(NCHUNK):
            t = c2pool.tile([P, JT_PER_CHUNK, P], BF16, name=f"c2bT{ci}")
            dma_engs[ci % 3].dma_start(
                t[:],
                d_c2bT[:, ci * JT_PER_CHUNK * P : (ci + 1) * JT_PER_CHUNK * P]
                .rearrange("k (t p) -> k t p", p=P),
            )
            t_c2bT.append(t)
        nc.sync.dma_start(t_c1s[:], d_c1s.rearrange("(t p) k -> p t k", p=P))
        nc.gpsimd.dma_start(t_c2s[:], d_c2s.rearrange("(t p) k -> p t k", p=P))

        # warm the ACT table set before the drain path needs it
        nc.scalar.activation(t_dummy[:], t_ones[:],
                             mybir.ActivationFunctionType.Identity, bias=0.0)

        # PE warm-up: keep HAM busy while inputs stream in
        pwarm = psum.tile([P, P], F32, tag="pcross", name="pwarm")
        for w in range(24):
            nc.tensor.matmul(pwarm[:], t_ident[:], t_ident[:],
                             start=(w == 0), stop=(w == 23))

        # ---- |c1_i|^2 (fp32, per shard row) ----
        for t in range(MTI):
            nc.vector.scalar_tensor_tensor(
                out=t_sttscratch[:],
                in0=t_c1s[:, t],
                scalar=1.0,
                in1=t_c1s[:, t],
                op0=mybir.AluOpType.mult,
                op1=mybir.AluOpType.mult,
                accum_out=t_sq1[:, t : t + 1],
            )

        # ---- cross matmuls (j on partitions) + dual-engine drain ----
        nd = na = 0
        zhalf = 0
        zt = None
        for t in range(NJT):
            pt = psum.tile([P, NI], F32, tag="pcross", name="pcross")
            lhsT = t_c2bT[t // JT_PER_CHUNK][:, t % JT_PER_CHUNK]
            for c in range(NI // 512):
                nc.tensor.matmul(
                    pt[:, c * 512 : (c + 1) * 512],
                    lhsT,
                    t_c1bT[:, c * 512 : (c + 1) * 512],
                    start=True,
                    stop=True,
                )
            bias = t_sq2neg[:, t : t + 1]
            if t in DVE_TILES:
                nc.vector.scalar_tensor_tensor(
                    out=t_zD[(nd + 1) % 2][:],
                    in0=pt[:],
                    scalar=bias,
                    in1=t_zD[nd % 2][:],
                    op0=mybir.AluOpType.add,
                    op1=mybir.AluOpType.max,
                )
                nd += 1
            else:
                if zhalf == 0:
                    zt = zring.tile([P, 2, NI], BF16, name="zt")
                nc.scalar.activation(
                    zt[:, zhalf], pt[:], mybir.ActivationFunctionType.Identity,
                    bias=bias, scale=1.0,
                )
                if zhalf == 1:
                    # one bf16 2x tensor_max folds both tiles of the pair
                    # into the two independent halves of the zA accumulator
                    nc.vector.tensor_max(t_zA[(na + 1) % 2][:],
                                         t_zA[na % 2][:], zt[:])
                    na += 1
                zhalf ^= 1
        if zhalf == 1:  # lone last ACT tile: pad its pair-half with -inf
            nc.gpsimd.memset(zt[:, 1], NEG_BIG)
            nc.vector.tensor_max(t_zA[(na + 1) % 2][:],
                                 t_zA[na % 2][:], zt[:])
            na += 1

        # ---- column stats: sum(c1), sum(c1^2), sum(c2 slice) (fp32 MMs) ----
        nc.scalar.activation(t_c1sq[:], t_c1s[:], mybir.ActivationFunctionType.Square)
        ps = psum.tile([1, 3, D], F32, tag="pcross", name="pstats")
        for t in range(MTI):
            nc.tensor.matmul(ps[:, 0], t_ones[:], t_c1s[:, t],
                             start=(t == 0), stop=(t == MTI - 1))
        for t in range(MTI):
            nc.tensor.matmul(ps[:, 1], t_ones[:], t_c1sq[:, t],
                             start=(t == 0), stop=(t == MTI - 1))
        for t in range(MTI):
            nc.tensor.matmul(ps[:, 2], t_ones[:], t_c2s[:, t],
                             start=(t == 0), stop=(t == MTI - 1))
        nc.vector.tensor_copy(t_souts[:], ps[:])
        nc.sync.dma_start(d_stats, t_souts[0])

        # ---- tail: combine partial maxes, partition-reduce via PE transpose
        nc.vector.tensor_max(t_zfin[:], t_zD[nd % 2][:], t_zA[na % 2][:, 0])
        nc.vector.tensor_max(t_zfin[:], t_zfin[:], t_zA[na % 2][:, 1])
        ptr = psum.tile([P, MTI, P], BF16, tag="pcross", name="ptr")
        for c in range(MTI):
            nc.tensor.transpose(ptr[:, c], t_zfin[:, c * P : (c + 1) * P],
                                t_ident[:])
        nc.vector.tensor_reduce(t_gmax[:], ptr[:], axis=mybir.AxisListType.X,
                                op=mybir.AluOpType.max)
        nc.sync.dma_start(d_gmax, t_gmax[:])
        nc.sync.dma_start(d_sq1, t_sq1[:])

    nc.compile()
    return nc


def _prep_inputs(cluster1: np.ndarray, cluster2: np.ndarray):
    """Host-side sharding + operand layout prep."""
    c2b = cluster2.astype(BF16_NP)
    c2bT = np.ascontiguousarray(c2b.T)                       # [128, 8192] bf16
    sq2 = (c2b.astype(np.float32) ** 2).sum(axis=1)          # [8192] fp32
    sq2neg = np.ascontiguousarray((-sq2).reshape(NJT, P).T).astype(np.float32)

    in_maps = []
    for c in range(N_CORES):
        c1s = np.ascontiguousarray(cluster1[c * NI : (c + 1) * NI])
        c2s = np.ascontiguousarray(cluster2[c * NI : (c + 1) * NI])
        c1bT = np.ascontiguousarray((2.0 * c1s).astype(BF16_NP).T)  # [128, 1024]
        in_maps.append({
            "c1s": c1s,
            "c2s": c2s,
            "c1bT": c1bT,
            "c2bT": c2bT,
            "sq2neg": sq2neg,
        })
    return in_maps


def _finish(results) -> np.float32:
    """Combine the 8 per-core partials into the scalar loss (host, fp64)."""
    dist_sum = 0.0
    s1 = np.zeros(D, np.float64)
    q1 = np.zeros(D, np.float64)
    s2 = np.zeros(D, np.float64)
    for r in results:
        gmax = np.asarray(r["gmax"], np.float64)   # [128, 8]; row = t*128+p
        sq1 = np.asarray(r["sq1"], np.float64)
        dist_sum += (sq1 - gmax).sum()
        stats = np.asarray(r["stats"], np.float64)  # [3, 128]
        s1 += stats[0]
        q1 += stats[1]
        s2 += stats[2]
    dist = dist_sum / N1
    m1 = s1 / N1
    m2 = s2 / N2
    mean_loss = ((m1 - m2) ** 2).mean()
    var = q1 / N1 - m1 ** 2
    disp = np.maximum(MIN_VARIANCE - var, 0.0).mean()
    return np.float32(mean_loss + dist + disp)


def _run(inputs, trace=False, **kwargs):
    """Run on the 8 NeuronCores. Returns (loss_scalar, BassKernelResults)."""
    if "nc" not in _cached:
        _cached["nc"] = _build_program()
    nc = _cached["nc"]
    in_maps = _prep_inputs(np.asarray(inputs["cluster1"], np.float32),
                           np.asarray(inputs["cluster2"], np.float32))
    res = run_bass_kernel_spmd(nc, in_maps, list(range(N_CORES)), trace=trace,
                               **kwargs)
    loss = _finish(res.results)
    return loss, res


def kernel(cluster1: np.ndarray, cluster2: np.ndarray) -> np.ndarray:
    loss, _ = _run({"cluster1": cluster1, "cluster2": cluster2})
    return np.asarray(loss, dtype=np.float32)


# revision 8
# speedup vs baseline: 1.2187x; 1.1053x over previous
"""Trainium2 Bass kernel for nn_CustomLoss_45449343926664 (retrieval_knn).

loss = mse(mean(c1), mean(c2))
     + mean_i min_j ||c1_i - c2_j||^2
     + mean_k relu(0.1 - var(c1)_k)

Sharding: cluster1 rows are data-parallel across the 8 cores (1024 rows
each); cluster2 is replicated. Each core computes its [1024, 8192] block
of the distance matrix on the tensor engine in bf16 (c1 pre-scaled by 2
so PSUM holds 2<c1,c2>), with matmuls in "j-on-partitions" orientation:
psum tile [128 j, 1024 i] per j-tile, so -|c2_j|^2 is a per-partition
bias. The 8192x8192-element drain/row-max is split across DVE and ACT:

  - DVE j-tiles: scalar_tensor_tensor fused drain
        zD' = max(psum + bias, zD)     (one 1x pass, ping-pong accum)
  - ACT j-tiles: activation(Identity, bias) -> z bf16 tile; PAIRS of z
    tiles are folded by one DVE bf16 tensor_max (2x mode, [128, 2048])
    into the two independent halves of the zA accumulator.
  - tail: zfin = max(zD, zA halves); 8 PE transposes + one 3D
    reduce_max give per-row max_j(2<c1_i,c2_j> - |c2_j|^2).

|c1_i|^2 (fp32) and the mean/variance column stats (fp32 ones-matmuls
accumulated in PSUM) are computed on device as well; the host only sums
the 8 tiny per-core partials (a few KB) into the final scalar.

Host-side input prep per core: slicing, the bf16 cast + transpose of the
matmul operands (layout prep), and |c2_j|^2 of the bf16-rounded c2 (32KB,
consistent with the bf16 cross term).
"""
import os
import sys

import numpy as np
import ml_dtypes

if os.path.isdir("/opt/trn_rl_repo") and "/opt/trn_rl_repo" not in sys.path:
    sys.path.insert(0, "/opt/trn_rl_repo")

from contextlib import ExitStack

import concourse.bass as bass
import concourse.tile as tile
from concourse import bacc, mybir
from concourse.bass_utils import run_bass_kernel_spmd
from concourse.masks import make_identity

F32 = mybir.dt.float32
BF16 = mybir.dt.bfloat16
BF16_NP = ml_dtypes.bfloat16
NEG_BIG = -3.0e38

N_CORES = 8
N1 = 8192            # cluster1 rows (total)
N2 = 8192            # cluster2 rows
D = 128              # feature dim = partition count
P = 128
NI = N1 // N_CORES   # 1024 c1 rows per core
MTI = NI // P        # 8 i-tiles of 128
NJT = N2 // P        # 64 j-tiles of 128
NCHUNK = 8           # c2bT DMA chunks
JT_PER_CHUNK = NJT // NCHUNK

# j-tiles drained by the fused DVE path (17 of 64); the other 47 go to
# ACT (one of them padded with -inf to make 24 fold-pairs).
DVE_TILES = {2, 6, 10, 14, 18, 22, 26, 30, 34, 38, 42, 46, 50, 54, 58, 61, 63}
MIN_VARIANCE = 0.1

_cached = {}


def _build_program():
    """Build + compile the single-core SPMD program (same for all cores)."""
    nc = bacc.Bacc(
        "TRN2",
        target_bir_lowering=False,
        debug=False,
        enable_asserts=False,
        num_devices=N_CORES,
    )

    d_c1s = nc.dram_tensor("c1s", [NI, D], F32, kind="ExternalInput").ap()
    d_c2s = nc.dram_tensor("c2s", [NI, D], F32, kind="ExternalInput").ap()
    d_c1bT = nc.dram_tensor("c1bT", [D, NI], BF16, kind="ExternalInput").ap()
    d_c2bT = nc.dram_tensor("c2bT", [D, N2], BF16, kind="ExternalInput").ap()
    d_sq2neg = nc.dram_tensor("sq2neg", [P, NJT], F32, kind="ExternalInput").ap()

    d_gmax = nc.dram_tensor("gmax", [P, MTI], F32, kind="ExternalOutput").ap()
    d_sq1 = nc.dram_tensor("sq1", [P, MTI], F32, kind="ExternalOutput").ap()
    d_stats = nc.dram_tensor("stats", [3, D], F32, kind="ExternalOutput").ap()

    with tile.TileContext(nc) as tc, ExitStack() as ctx:
        const = ctx.enter_context(tc.tile_pool(name="const", bufs=1))
        c2pool = ctx.enter_context(tc.tile_pool(name="c2pool", bufs=NCHUNK))
        zring = ctx.enter_context(tc.tile_pool(name="zring", bufs=6))
        psum = ctx.enter_context(tc.tile_pool(name="psum", bufs=4, space="PSUM"))

        t_c1s = const.tile([P, MTI, P], F32)
        t_c2s = const.tile([P, MTI, P], F32)
        t_c1bT = const.tile([P, NI], BF16)
        t_sq2neg = const.tile([P, NJT], F32)
        t_ones = const.tile([P, 1], F32)
        t_sq1 = const.tile([P, MTI], F32)
        t_zA = [const.tile([P, 2, NI], BF16, name=f"zA{i}") for i in range(2)]
        t_zD = [const.tile([P, NI], BF16, name=f"zD{i}") for i in range(2)]
        t_zfin = const.tile([P, NI], BF16)
        t_gmax = const.tile([P, MTI], F32)
        t_c1sq = const.tile([P, MTI, P], F32)
        t_souts = const.tile([1, 3, D], F32)
        t_sttscratch = const.tile([P, P], F32)
        t_ident = const.tile([P, P], BF16)
        t_dummy = const.tile([P, 1], F32)

        # identity early (gpsimd) so PE warm-up matmuls can start during loads
        make_identity(nc, t_ident[:])
        nc.gpsimd.memset(t_zA[0][:], NEG_BIG)
        nc.gpsimd.memset(t_zD[0][:], NEG_BIG)
        nc.vector.memset(t_ones[:], 1.0)

        # ---- input DMAs, spread across the three DGE-capable engines ----
        nc.scalar.dma_start(t_c1bT[:], d_c1bT)
        nc.sync.dma_start(t_sq2neg[:], d_sq2neg)
        t_c2bT = []
        dma_engs = [nc.sync, nc.gpsimd]
        for ci in range(NCHUNK):
            t = c2pool.tile([P, JT_PER_CHUNK, P], BF16, name=f"c2bT{ci}")
            dma_engs[ci % 2].dma_start(
                t[:],
                d_c2bT[:, ci * JT_PER_CHUNK * P : (ci + 1) * JT_PER_CHUNK * P]
                .rearrange("k (t p) -> k t p", p=P),
            )
            t_c2bT.append(t)
        nc.scalar.dma_start(t_c1s[:], d_c1s.rearrange("(t p) k -> p t k", p=P))
        nc.scalar.dma_start(t_c2s[:], d_c2s.rearrange("(t p) k -> p t k", p=P))

        # warm the ACT table set before the drain path needs it
        nc.scalar.activation(t_dummy[:], t_ones[:],
                             mybir.ActivationFunctionType.Identity, bias=0.0)

        # PE warm-up: keep HAM busy while inputs stream in
        pwarm = psum.tile([P, P], F32, tag="pcross", name="pwarm")
        for w in range(24):
            nc.tensor.matmul(pwarm[:], t_ident[:], t_ident[:],
                             start=(w == 0), stop=(w == 23))

        # ---- |c1_i|^2 (fp32, per shard row) ----
        for t in range(MTI):
            nc.vector.scalar_tensor_tensor(
                out=t_sttscratch[:],
                in0=t_c1s[:, t],
                scalar=1.0,
                in1=t_c1s[:, t],
                op0=mybir.AluOpType.mult,
                op1=mybir.AluOpType.mult,
                accum_out=t_sq1[:, t : t + 1],
            )

        # ---- cross matmuls (j on partitions) + dual-engine drain ----
        nd = na = 0
        zhalf = 0
        zt = None
        for t in range(NJT):
            pt = psum.tile([P, NI], F32, tag="pcross", name="pcross")
            lhsT = t_c2bT[t // JT_PER_CHUNK][:, t % JT_PER_CHUNK]
            for c in range(NI // 512):
                nc.tensor.matmul(
                    pt[:, c * 512 : (c + 1) * 512],
                    lhsT,
                    t_c1bT[:, c * 512 : (c + 1) * 512],
                    start=True,
                    stop=True,
                )
            bias = t_sq2neg[:, t : t + 1]
            if t in DVE_TILES:
                nc.vector.scalar_tensor_tensor(
                    out=t_zD[(nd + 1) % 2][:],
                    in0=pt[:],
                    scalar=bias,
                    in1=t_zD[nd % 2][:],
                    op0=mybir.AluOpType.add,
                    op1=mybir.AluOpType.max,
                )
                nd += 1
            else:
                if zhalf == 0:
                    zt = zring.tile([P, 2, NI], BF16, name="zt")
                nc.scalar.activation(
                    zt[:, zhalf], pt[:], mybir.ActivationFunctionType.Identity,
                    bias=bias, scale=1.0,
                )
                if zhalf == 1:
                    # one bf16 2x tensor_max folds both tiles of the pair
                    # into the two independent halves of the zA accumulator
                    nc.vector.tensor_max(t_zA[(na + 1) % 2][:],
                                         t_zA[na % 2][:], zt[:])
                    na += 1
                zhalf ^= 1
        if zhalf == 1:  # lone last ACT tile: pad its pair-half with -inf
            nc.gpsimd.memset(zt[:, 1], NEG_BIG)
            nc.vector.tensor_max(t_zA[(na + 1) % 2][:],
                                 t_zA[na % 2][:], zt[:])
            na += 1

        # ---- column stats: sum(c1), sum(c1^2), sum(c2 slice) (fp32 MMs) ----
        nc.scalar.activation(t_c1sq[:], t_c1s[:], mybir.ActivationFunctionType.Square)
        ps = psum.tile([1, 3, D], F32, tag="pcross", name="pstats")
        for t in range(MTI):
            nc.tensor.matmul(ps[:, 0], t_ones[:], t_c1s[:, t],
                             start=(t == 0), stop=(t == MTI - 1))
        for t in range(MTI):
            nc.tensor.matmul(ps[:, 1], t_ones[:], t_c1sq[:, t],
                             start=(t == 0), stop=(t == MTI - 1))
        for t in range(MTI):
            nc.tensor.matmul(ps[:, 2], t_ones[:], t_c2s[:, t],
                             start=(t == 0), stop=(t == MTI - 1))
        nc.vector.tensor_copy(t_souts[:], ps[:])
        nc.sync.dma_start(d_stats, t_souts[0])

        # ---- tail: combine partial maxes, partition-reduce via PE transpose
        nc.vector.tensor_max(t_zfin[:], t_zD[nd % 2][:], t_zA[na % 2][:, 0])
        nc.vector.tensor_max(t_zfin[:], t_zfin[:], t_zA[na % 2][:, 1])
        ptr = psum.tile([P, MTI, P], BF16, tag="pcross", name="ptr")
        for c in range(MTI):
            nc.tensor.transpose(ptr[:, c], t_zfin[:, c * P : (c + 1) * P],
                                t_ident[:])
        nc.vector.tensor_reduce(t_gmax[:], ptr[:], axis=mybir.AxisListType.X,
                                op=mybir.AluOpType.max)
        nc.sync.dma_start(d_gmax, t_gmax[:])
        nc.sync.dma_start(d_sq1, t_sq1[:])

    nc.compile()
    return nc


def _prep_inputs(cluster1: np.ndarray, cluster2: np.ndarray):
    """Host-side sharding + operand layout prep."""
    c2b = cluster2.astype(BF16_NP)
    c2bT = np.ascontiguousarray(c2b.T)                       # [128, 8192] bf16
    sq2 = (c2b.astype(np.float32) ** 2).sum(axis=1)          # [8192] fp32
    sq2neg = np.ascontiguousarray((-sq2).reshape(NJT, P).T).astype(np.float32)

    in_maps = []
    for c in range(N_CORES):
        c1s = np.ascontiguousarray(cluster1[c * NI : (c + 1) * NI])
        c2s = np.ascontiguousarray(cluster2[c * NI : (c + 1) * NI])
        c1bT = np.ascontiguousarray((2.0 * c1s).astype(BF16_NP).T)  # [128, 1024]
        in_maps.append({
            "c1s": c1s,
            "c2s": c2s,
            "c1bT": c1bT,
            "c2bT": c2bT,
            "sq2neg": sq2neg,
        })
    return in_maps


def _finish(results) -> np.float32:
    """Combine the 8 per-core partials into the scalar loss (host, fp64)."""
    dist_sum = 0.0
    s1 = np.zeros(D, np.float64)
    q1 = np.zeros(D, np.float64)
    s2 = np.zeros(D, np.float64)
    for r in results:
        gmax = np.asarray(r["gmax"], np.float64)   # [128, 8]; row = t*128+p
        sq1 = np.asarray(r["sq1"], np.float64)
        dist_sum += (sq1 - gmax).sum()
        stats = np.asarray(r["stats"], np.float64)  # [3, 128]
        s1 += stats[0]
        q1 += stats[1]
        s2 += stats[2]
    dist = dist_sum / N1
    m1 = s1 / N1
    m2 = s2 / N2
    mean_loss = ((m1 - m2) ** 2).mean()
    var = q1 / N1 - m1 ** 2
    disp = np.maximum(MIN_VARIANCE - var, 0.0).mean()
    return np.float32(mean_loss + dist + disp)


def _run(inputs, trace=False, **kwargs):
    """Run on the 8 NeuronCores. Returns (loss_scalar, BassKernelResults)."""
    if "nc" not in _cached:
        _cached["nc"] = _build_program()
    nc = _cached["nc"]
    in_maps = _prep_inputs(np.asarray(inputs["cluster1"], np.float32),
                           np.asarray(inputs["cluster2"], np.float32))
    res = run_bass_kernel_spmd(nc, in_maps, list(range(N_CORES)), trace=trace,
                               **kwargs)
    loss = _finish(res.results)
    return loss, res


def kernel(cluster1: np.ndarray, cluster2: np.ndarray) -> np.ndarray:
    loss, _ = _run({"cluster1": cluster1, "cluster2": cluster2})
    return np.asarray(loss, dtype=np.float32)
